# revision 36
# baseline (speedup 1.0000x reference)
"""CTC loss kernel for Trainium2 (Bass/Tile), 8-core data-parallel.

Per core (8 samples): linear-space CTC forward recurrence, scanned
column-by-column over the extended-label axis (S=201).  The time axis
(T=1000) lives on the free dim, split into 4 chunks of 250 mapped to the
four SBUF partition quadrants (partition = 32*chunk + sample).  Each
column is computed with a two-pass blocked scan: ONE full-width
zero-init tensor_tensor_scan covers all 4 chunks at once (partitions are
free parallelism), a second full-width scan forms the chunk prefix
products of q (with the cross-chunk scale conversion RB folded into its
initial value), a 3-step tiny carry recurrence links the chunk
boundaries, and one full-width fused multiply-add reconstructs
x = z + carry * P'.  Odd columns add one fused C-op (even columns have
skip==0 structurally, so the scans read the previous column's tile
directly).

Numerics: per-frame normalizer nu[t] = (1.2/(2l+1)) * sum_s y_pred[t,ext[s]]
(folded into the w matmul vector host-side) keeps drift to a random walk;
per-(sample,chunk) rescales every 16 columns (never scaling up, rho
exponent clamped, Ln computed with a 2^-32 prescale) keep everything in
f32; the final loss re-adds the log-nu prefix sum (N8) and the
accumulated log scales (lambda).  Matmul inputs and the q-hat DRAM
bounce are bf16 (the nu reciprocals are exact powers of two, so the
normalizer path stays exact); the scan state itself stays f32.

Host side does only label-index bookkeeping (one-hot gather matrices,
masks) plus a pure layout transpose of y_pred; all y_pred-dependent math
runs on device.
"""
import os
import sys

sys.path.insert(0, "/opt/trn_rl_repo")

import numpy as np

import concourse.bass as bass
import concourse.bacc as bacc
import concourse.mybir as mybir
import concourse.tile as tile
from concourse.bass_utils import run_bass_kernel_spmd

B, T, C, L = 64, 1000, 128, 100
S = 2 * L + 1            # 201
NB = 8                   # samples per core
NCORE = 8
NCH, TC = 4, 250         # time chunks x chunk length
RS = 16                  # rescale every RS columns
SBLK = 24                # columns per streamed block
KAPPA = 1.2              # normalizer constant (per-sample cK = KAPPA/(2l+1))
LN232 = 22.18070977791825   # 32*ln(2)
LN2 = 0.6931471805599453
EXPMASK = 0x7F800000
RCPBASE = 0x7F000000
F32 = mybir.dt.float32
BF16 = mybir.dt.bfloat16
INT32 = mybir.dt.int32
AOP = mybir.AluOpType
AFT = mybir.ActivationFunctionType

# big tensor column offsets (partition dim = C = 128), dtype bf16
YP0 = 0                  # ypT: col b*1000 + t
G0 = NB * T              # g:  col G0 + b*201 + s
W0 = G0 + NB * S         # w:  col W0 + b
SELB0 = W0 + NB          # sel8: 8 blocks of 128 (replication matmul weights)
WB0 = SELB0 + NB * 128   # wb: 8 blocks of 8 (w_b in column b, else zero)
BIGN = WB0 + NB * NB

# aux tensor column offsets (partition dim = 128), dtype f32
SKP0 = 0                 # skipm [128, S]
EM0 = SKP0 + S           # emask [128, TC+1]
VE0 = EM0 + TC + 1       # veps [128, 2*NB]
I00 = VE0 + 2 * NB       # ind0 [128, 2*NB]
SEL0 = I00 + 2 * NB      # sel [128, NB]
KB0 = SEL0 + NB          # per-sample N8 offset const [rows 0:8, 1]
TM0 = KB0 + 1            # tmask [rows 0:8, T]
AUXN = TM0 + T

_cached = {}


def _build_program():
    from contextlib import ExitStack

    nc = bacc.Bacc(None, target_bir_lowering=False)

    big_d = nc.dram_tensor("big", [C, BIGN], BF16, kind="ExternalInput")
    aux_d = nc.dram_tensor("aux", [128, AUXN], F32, kind="ExternalInput")
    loss_d = nc.dram_tensor("loss", [NB, 1], F32, kind="ExternalOutput")
    dbg_d = nc.dram_tensor("dbg", [4, 128], F32, kind="ExternalOutput")
    # internal bounce, split per sig so the scan's early reads only wait on
    # the sig0 writes
    qhat_ds = (nc.dram_tensor("qhat0", [NB, 128, T], BF16),
               nc.dram_tensor("qhat1", [NB, S - 128, T], BF16))

    PS_SIG = (128, S - 128)
    HB = 2                    # samples per gather PSUM round

    with tile.TileContext(nc) as tc, ExitStack() as ctx:
        pers = ctx.enter_context(tc.tile_pool(name="pers", bufs=1))
        pbig = ctx.enter_context(tc.tile_pool(name="pbig", bufs=2, space="PSUM"))
        psml = ctx.enter_context(tc.tile_pool(name="psml", bufs=2, space="PSUM"))
        qblk_pool = ctx.enter_context(tc.tile_pool(name="qblk", bufs=2))
        vrow = ctx.enter_context(tc.tile_pool(name="vrow", bufs=2))

        big = pers.tile([C, BIGN], BF16, tag="big")
        aux = pers.tile([128, AUXN], F32, tag="aux")
        n8parts = pers.tile([NB, 2], F32, tag="n8parts")
        rnusAll = pers.tile([NB, 2 * 500], BF16, tag="rnusAll")
        nurepS = pers.tile([128, 2, NB, 500], F32, tag="nurepS")
        stags = [pers.tile([128, NB, 500], BF16, tag=f"stag{i}",
                           name=f"stag{i}") for i in range(2)]
        junk = pers.tile([128, 501], F32, tag="junk")
        X = []
        for i in range(3):
            xt = pers.tile([128, TC + 1], F32, tag=f"X{i}", name=f"X{i}")
            X.append(xt)
        Cbuf = pers.tile([128, TC], F32, tag="Cbuf")
        sc = pers.tile([128, 8], F32, tag="sc")
        LAM, RHO, LSH, MRE, TMP, R199, R200, RTOT = range(8)
        sci = pers.tile([128, 8], INT32, tag="sci")
        LAMI, LSHI, DI, EI, RA, RB = range(6)
        lamF = pers.tile([128, 1], F32, tag="lamF")
        v8 = pers.tile([NB, 4], F32, tag="v8")
        N8c, LOGRc, T1c, LOSSc = range(4)

        def ypr(b, h):
            return big[:, YP0 + b * T + h * 500:YP0 + b * T + (h + 1) * 500]

        # ---- loads ----
        # weights/gather region first, then ypT in per-(b,h) slices so each
        # nu matmul starts as soon as its own slice lands
        nc.sync.dma_start(aux[:], aux_d[:])
        nc.sync.dma_start(big[:, G0:BIGN], big_d[:, G0:BIGN])
        for h in range(2):
            for b in range(NB):
                lo = YP0 + b * T + h * 500
                nc.sync.dma_start(big[:, lo:lo + 500], big_d[:, lo:lo + 500])

        # ---- nu phase (batched over the 8 samples) ----
        # accumulate all 8 samples' nu rows into one [8,500] PSUM tile per
        # half via one-hot-column weight blocks; the exponent chain then runs
        # batched on DVE straight out of PSUM.
        nui = vrow.tile([NB, 2 * 500], INT32, tag="nui_t")
        ef = vrow.tile([NB, 2 * 500], F32, tag="ef_t")
        tms = aux[0:NB, TM0:TM0 + T]
        for h in range(2):
            pnuh = psml.tile([NB, 512], F32, tag="psm")
            for b in range(NB):
                nc.tensor.matmul(pnuh[0:NB, 0:500],
                                 big[:, WB0 + b * NB:WB0 + (b + 1) * NB],
                                 ypr(b, h), start=(b == 0), stop=(b == NB - 1))
            # exponent bits of nu (power-of-two normalizer)
            nc.vector.tensor_scalar(nui[:, h * 500:(h + 1) * 500],
                                    pnuh[0:NB, 0:500].bitcast(INT32),
                                    EXPMASK, None, AOP.bitwise_and)
        # exact reciprocal: bits = RCPBASE - expbits
        nc.vector.tensor_scalar(ef.bitcast(INT32)[:], nui[:],
                                -1, RCPBASE, AOP.mult, AOP.add)
        # rnus (bf16, exact powers of two) masked by tmask
        nc.vector.tensor_tensor(rnusAll[:], ef[:], tms, AOP.mult)
        # E field as f32 for the N8 sum
        nc.vector.tensor_scalar(nui[:], nui[:],
                                23, None, AOP.logical_shift_right)
        nc.vector.tensor_copy(ef[:], nui[:])
        nc.vector.scalar_tensor_tensor(
            junk[0:NB, 0:500], ef[:, 0:500], 1.0, tms[:, 0:500],
            AOP.mult, AOP.mult, accum_out=n8parts[:, 0:1])
        nc.vector.scalar_tensor_tensor(
            junk[0:NB, 0:500], ef[:, 500:1000], 1.0, tms[:, 500:1000],
            AOP.mult, AOP.mult, accum_out=n8parts[:, 1:2])

        # rnus replication across all 128 partitions happens inline in the
        # sig0 gather below, so the first gather STT is not gated by all 16
        # replication matmuls
        def emit_repl(b, h):
            prep = psml.tile([128, 512], F32, tag="prep")
            nc.tensor.matmul(prep[:, 0:500],
                             big[0:NB, SELB0 + b * 128:SELB0 + (b + 1) * 128],
                             rnusAll[:, h * 500:(h + 1) * 500],
                             start=True, stop=True)
            nc.scalar.copy(nurepS[:, h, b, :], prep[:, 0:500])

        # ---- gather phases ----
        def emit_gather(sig):
            ps = PS_SIG[sig]
            s0 = 0 if sig == 0 else 128
            for h in range(2):
                for hb in range(NB // HB):
                    if sig == 0:
                        for bb in range(HB):
                            emit_repl(hb * HB + bb, h)
                    gat = pbig.tile([128, HB, 512], F32, tag="gat")
                    for bb in range(HB):
                        b = hb * HB + bb
                        nc.tensor.matmul(
                            gat[0:ps, bb, 0:500],
                            big[:, G0 + b * S + s0:G0 + b * S + s0 + ps],
                            ypr(b, h), start=True, stop=True)
                    for bb in range(HB):
                        b = hb * HB + bb
                        nc.vector.scalar_tensor_tensor(
                            stags[h][0:ps, b, :], gat[0:ps, bb, 0:500],
                            aux[0:ps, VE0 + NB * sig + b:VE0 + NB * sig + b + 1],
                            nurepS[0:ps, h, b, :],
                            AOP.add, AOP.mult)
                if h == 0:
                    nc.vector.tensor_tensor(
                        stags[0][0:ps, :, 0], stags[0][0:ps, :, 0],
                        aux[0:ps, I00 + NB * sig:I00 + NB * (sig + 1)],
                        AOP.mult)
                nc.sync.dma_start(
                    qhat_ds[sig][:, :, h * 500:(h + 1) * 500]
                    .rearrange("b s j -> s b j"),
                    stags[h][0:ps, :, :])

        emit_gather(0)

        # preload the Ln activation table while the Activation engine is
        # idle, so the finalize's Ln does not pay the table load
        actwarm = pers.tile([1, 1], F32, tag="actwarm")
        nc.gpsimd.memset(actwarm[:], 1.0)
        nc.scalar.activation(actwarm[:], actwarm[:], AFT.Ln)

        # ---- scan phase (two-pass per column) ----
        # Pass 1: ONE full-width zero-init scan covers all 4 time chunks
        # (partitions are free parallelism).  Pass 2: x = z + carry * P'
        # where P' is the chunk prefix-product of q with the carry scale
        # conversion RB folded into the scan's initial value, and the
        # carries follow a tiny 3-step recurrence.  This replaces 4 chained
        # 250-long scans with 2 full-width scans + 1 full-width STT.
        zb = pers.tile([128, TC + 1], F32, tag="zb")
        Pb = [pers.tile([128, TC + 1], F32, tag=f"Pb{i}", name=f"Pb{i}")
              for i in range(2)]
        ones250 = pers.tile([128, TC], F32, tag="ones250")
        cvec = pers.tile([128, 1], F32, tag="cvec")
        rb01 = pers.tile([128, 1], F32, tag="rb01")
        RCPI = 6              # sci scratch column for the rescale reciprocal
        for i in range(3):
            nc.gpsimd.memset(X[i][:], 0.0)
            nc.gpsimd.memset(X[i][0:NB, 0:1], 1.0)
        nc.gpsimd.memset(ones250[:], 1.0)
        nc.gpsimd.memset(zb[:, 0:1], 0.0)
        nc.gpsimd.memset(sc[:], 0.0)
        nc.gpsimd.memset(sci[:], 0)
        nc.gpsimd.memset(sci[:, RB:RB + 1], 0x3F800000)
        # chunk 0 has no incoming carry: P'[0:NB] = 0 makes the full-width
        # fixup a no-op there; its cvec slot holds the 1.0 virtual-init seed
        nc.gpsimd.memset(sci[0:NB, RB:RB + 1], 0)
        nc.gpsimd.memset(cvec[:], 0.0)
        nc.gpsimd.memset(cvec[0:NB, 0:1], 1.0)
        nc.gpsimd.memset(rb01[:], 1.0)

        sblocks = []
        s = 0
        while s < S:
            n = min(SBLK, S - s)
            if s < 128 < s + n:
                n = 128 - s          # align a block boundary at the sig split
            if S - (s + n) == 1:
                n += 1
            sblocks.append((s, n))
            s += n
        col2bi = {}
        for bi, (sb, nsb) in enumerate(sblocks):
            for k in range(nsb):
                col2bi[sb + k] = bi

        rbF = sci.bitcast(F32)
        plam8_holder = []
        # two persistent ping-pong q tiles; the full-width scans read every
        # partition, so zero the unused lanes once up front (the DMAs only
        # ever write the real lanes, so they stay zero)
        max_nsb = max(n for _, n in sblocks)
        qtiles = [pers.tile([128, max_nsb, TC], BF16, tag=f"qt{i}",
                            name=f"qt{i}") for i in range(2)]
        for qt in qtiles:
            nc.gpsimd.memset(qt[:], 0.0)

        def load_block(bi):
            sb, nsb = sblocks[bi]
            qblk = qtiles[bi % 2]
            sig = 0 if sb < 128 else 1
            qsrc = qhat_ds[sig]
            qs0 = sb - (0 if sig == 0 else 128)
            for c in range(NCH):
                nc.sync.dma_start(
                    qblk[32 * c:32 * c + NB, 0:nsb, :],
                    qsrc[:, qs0:qs0 + nsb, c * TC:(c + 1) * TC])

        def qop(s):
            bi = col2bi[s]
            return qtiles[bi % 2], s - sblocks[bi][0]

        def emit_P(s):
            qb, kk = qop(s)
            nc.vector.tensor_tensor_scan(
                Pb[s % 2][:, 1:TC + 1], ones250[:], qb[:, kk, :],
                rbF[:, RB:RB + 1], AOP.mult, AOP.mult)

        load_block(0)
        emit_P(0)
        for bi, (sb, nsb) in enumerate(sblocks):
            if bi == 1:
                # sig1's gather rides behind the first block's scan columns:
                # its data is only needed ~5 blocks later
                emit_gather(1)
            if bi + 1 < len(sblocks):
                load_block(bi + 1)      # one block of DMA lookahead
            for k in range(nsb):
                s = sb + k
                xs = X[s % 3]
                xm1 = X[(s + 2) % 3]
                xm2 = X[(s + 1) % 3]
                qb, kk = qop(s)
                Pcur = Pb[s % 2]
                if s % 2 == 1:
                    # odd columns: C = xm1 + skip*xm2 (skip can be nonzero)
                    nc.vector.scalar_tensor_tensor(
                        Cbuf[:], xm2[:, 0:TC], aux[:, SKP0 + s:SKP0 + s + 1],
                        xm1[:, 0:TC], AOP.mult, AOP.add)
                    d0 = Cbuf
                else:
                    # even columns are blanks: skip == 0 for every sample, so
                    # C = xm1 and the scan reads the previous column directly
                    d0 = xm1
                # pass 1: zero-init scans of all 4 chunks, one instruction
                nc.vector.tensor_tensor_scan(
                    zb[:, 1:TC + 1], d0[:, 0:TC], qb[:, kk, :],
                    0.0, AOP.add, AOP.mult)
                epoch = (s + 1) % RS == 0 and s < 198
                if s + 1 < S and not epoch:
                    emit_P(s + 1)       # filler: hides carry-chain latency
                # tiny carry recurrence on RAW (unconverted) carries:
                # c[n+1] = c[n]*P'(end) + z(end) = x[n](end) in frame n
                for c in range(NCH - 1):
                    lo = 32 * c
                    nc.vector.scalar_tensor_tensor(
                        cvec[lo + 32:lo + 40, 0:1],
                        Pcur[lo:lo + NB, TC:TC + 1],
                        cvec[lo:lo + NB, 0:1],
                        zb[lo:lo + NB, TC:TC + 1],
                        AOP.mult, AOP.add)
                # chunk-boundary values for the next columns' C reads, in
                # each destination chunk's scale frame (seed lanes stay 1.0)
                nc.vector.tensor_tensor(xs[:, 0:1], cvec[:], rb01[:],
                                        AOP.mult)
                # pass 2 fixup: x = z + carry * P'
                nc.vector.scalar_tensor_tensor(
                    xs[:, 1:TC + 1], Pcur[:, 1:TC + 1], cvec[:, 0:1],
                    zb[:, 1:TC + 1], AOP.mult, AOP.add)
                if s in (199, 200):
                    rcol = R199 if s == 199 else R200
                    nc.vector.scalar_tensor_tensor(
                        junk[:, 0:TC + 1], xs[:], 1.0,
                        aux[:, EM0:EM0 + TC + 1],
                        AOP.mult, AOP.mult, accum_out=sc[:, rcol:rcol + 1])
                if s == 193:
                    # lambda is final after the last epoch (col 191): move
                    # its readout matmul off the serial finalize tail
                    nc.vector.tensor_copy(lamF[:], sci[:, LAMI:LAMI + 1])
                    nc.vector.tensor_scalar_mul(lamF[:], lamF[:], LN2)
                    plam8 = psml.tile([NB, 512], F32, tag="prep")
                    nc.tensor.matmul(plam8[:, 0:1], aux[:, SEL0:SEL0 + NB],
                                     lamF[:], start=True, stop=True)
                    plam8_holder.append(plam8)
                if epoch:
                    nc.vector.tensor_reduce(
                        sc[:, MRE:MRE + 1], xs[:], mybir.AxisListType.X,
                        AOP.max, apply_absolute_value=True)
                    nc.vector.tensor_scalar_max(
                        sc[:, MRE:MRE + 1], sc[:, MRE:MRE + 1], 1.0)
                    # exponent-bit games: exact power-of-two rescale
                    nc.vector.tensor_scalar(
                        sci[:, RA:RA + 1], sc[:, MRE:MRE + 1].bitcast(INT32),
                        EXPMASK, None, AOP.bitwise_and)
                    nc.vector.tensor_scalar(
                        sci[:, RCPI:RCPI + 1], sci[:, RA:RA + 1],
                        -1, RCPBASE, AOP.mult, AOP.add)
                    rcpf = sci.bitcast(F32)[:, RCPI:RCPI + 1]
                    nc.vector.tensor_scalar_mul(xs[:], xs[:], rcpf)
                    nc.vector.tensor_scalar_mul(xm1[:], xm1[:], rcpf)
                    # restore the virtual-init seeds (DVE, not Pool memset,
                    # to avoid two cross-engine round trips mid-epoch)
                    nc.vector.tensor_scalar(xs[0:NB, 0:1], xs[0:NB, 0:1],
                                            0.0, 1.0, AOP.mult, AOP.add)
                    nc.vector.tensor_scalar(xm1[0:NB, 0:1], xm1[0:NB, 0:1],
                                            0.0, 1.0, AOP.mult, AOP.add)
                    nc.vector.tensor_scalar(
                        sci[:, EI:EI + 1], sci[:, RA:RA + 1],
                        23, None, AOP.logical_shift_right)
                    nc.vector.tensor_scalar(
                        sci[:, EI:EI + 1], sci[:, EI:EI + 1],
                        127, None, AOP.subtract)
                    nc.vector.tensor_tensor(sci[:, LAMI:LAMI + 1],
                                            sci[:, LAMI:LAMI + 1],
                                            sci[:, EI:EI + 1], AOP.add)
                    nc.vector.tensor_copy(sci[32:64, LSHI:LSHI + 1],
                                          sci[0:32, LAMI:LAMI + 1])
                    nc.vector.tensor_copy(sci[64:96, LSHI:LSHI + 1],
                                          sci[32:64, LAMI:LAMI + 1])
                    nc.vector.tensor_copy(sci[96:128, LSHI:LSHI + 1],
                                          sci[64:96, LAMI:LAMI + 1])
                    nc.vector.tensor_tensor(sci[:, DI:DI + 1],
                                            sci[:, LSHI:LSHI + 1],
                                            sci[:, LAMI:LAMI + 1],
                                            AOP.subtract)
                    nc.vector.tensor_scalar(sci[:, DI:DI + 1],
                                            sci[:, DI:DI + 1],
                                            126, -126, AOP.min, AOP.max)
                    nc.vector.tensor_scalar(sci[:, RB:RB + 1],
                                            sci[:, DI:DI + 1],
                                            127, None, AOP.add)
                    nc.vector.tensor_scalar(sci[:, RB:RB + 1],
                                            sci[:, RB:RB + 1],
                                            23, None, AOP.logical_shift_left)
                    # chunk-0 lanes must keep P' == 0 (no incoming carry)
                    nc.vector.tensor_scalar(sci[0:NB, RB:RB + 1],
                                            sci[0:NB, RB:RB + 1],
                                            0, None, AOP.mult)
                    # boundary-conversion vector: RB everywhere except the
                    # chunk-0 seed lanes, which stay exactly 1.0
                    nc.vector.tensor_copy(rb01[:], rbF[:, RB:RB + 1])
                    nc.vector.tensor_scalar(rb01[0:NB, 0:1], rb01[0:NB, 0:1],
                                            0.0, 1.0, AOP.mult, AOP.add)
                    emit_P(s + 1)       # uses the refreshed RB

        # ---- finalize ----
        nc.vector.tensor_tensor(sc[:, RTOT:RTOT + 1], sc[:, R199:R199 + 1],
                                sc[:, R200:R200 + 1], AOP.add)
        pr8 = psml.tile([NB, 512], F32, tag="psm")
        nc.tensor.matmul(pr8[:, 0:1], aux[:, SEL0:SEL0 + NB],
                         sc[:, RTOT:RTOT + 1], start=True, stop=True)
        plam8 = plam8_holder[0]
        nc.vector.tensor_tensor(v8[:, N8c:N8c + 1], n8parts[:, 0:1],
                                n8parts[:, 1:2], AOP.add)
        nc.vector.scalar_tensor_tensor(
            v8[:, N8c:N8c + 1], v8[:, N8c:N8c + 1], LN2,
            aux[0:NB, KB0:KB0 + 1], AOP.mult, AOP.add)
        # split r = m * 2^(E-127), m in [1,2): exact exponent, Ln on mantissa
        ri8 = pers.tile([NB, 2], INT32, tag="ri8")
        rf8 = pers.tile([NB, 2], F32, tag="rf8")
        nc.vector.tensor_scalar(ri8[:, 0:1], pr8[:, 0:1].bitcast(INT32),
                                23, None, AOP.logical_shift_right)
        nc.vector.tensor_copy(rf8[:, 0:1], ri8[:, 0:1])
        nc.vector.tensor_scalar(ri8[:, 1:2], pr8[:, 0:1].bitcast(INT32),
                                0x007FFFFF, 0x3F800000,
                                AOP.bitwise_and, AOP.bitwise_or)
        nc.scalar.activation(v8[:, LOGRc:LOGRc + 1],
                             ri8.bitcast(F32)[:, 1:2], AFT.Ln)
        nc.vector.tensor_scalar(rf8[:, 0:1], rf8[:, 0:1],
                                127.0, LN2, AOP.subtract, AOP.mult)
        nc.vector.tensor_tensor(v8[:, LOGRc:LOGRc + 1],
                                v8[:, LOGRc:LOGRc + 1],
                                rf8[:, 0:1], AOP.add)
        nc.vector.tensor_tensor(v8[:, T1c:T1c + 1], v8[:, LOGRc:LOGRc + 1],
                                v8[:, N8c:N8c + 1], AOP.add)
        nc.vector.scalar_tensor_tensor(
            v8[:, LOSSc:LOSSc + 1], v8[:, T1c:T1c + 1], -1.0, plam8[:, 0:1],
            AOP.mult, AOP.subtract)
        nc.sync.dma_start(loss_d[:], v8[:, LOSSc:LOSSc + 1])

    nc.finalize()
    return nc


def _host_prep(y_true, y_pred, input_lengths, label_lengths):
    bf16 = mybir.dt.np(BF16)
    in_maps = []
    for core in range(NCORE):
        bsl = slice(core * NB, (core + 1) * NB)
        yt = y_true[bsl]
        ilen = input_lengths[bsl].astype(np.int64)
        llen = label_lengths[bsl].astype(np.int64)

        big = np.zeros((C, BIGN), np.float32)
        big[:, YP0:YP0 + NB * T] = (
            y_pred[bsl].transpose(2, 0, 1).reshape(C, NB * T))
        aux = np.zeros((128, AUXN), np.float32)

        for b in range(NB):
            l = int(llen[b]); o = 200 - 2 * l
            ext = np.full(S, -1, np.int32)
            for k in range(2 * l + 1):
                ext[o + k] = C - 1 if k % 2 == 0 else yt[b, (k - 1) // 2]
            gb = np.zeros((C, S), np.float32)
            for s in range(S):
                if ext[s] >= 0:
                    gb[ext[s], s] = 1.0
                k = s - o
                if k >= 2 and k % 2 == 1 and ext[s] != ext[s - 2]:
                    for c in range(NCH):
                        aux[32 * c + b, SKP0 + s] = 1.0
            big[:, G0 + b * S:G0 + (b + 1) * S] = gb
            wvec = gb.sum(axis=1) * np.float32(KAPPA * np.sqrt(2.0) / (2 * l + 1))
            big[:, W0 + b] = wvec
            big[b, SELB0 + b * 128:SELB0 + (b + 1) * 128] = 1.0
            big[:, WB0 + b * NB + b] = wvec
            for sig in range(2):
                s0, ps = (0, 128) if sig == 0 else (128, S - 128)
                for sp in range(ps):
                    if ext[s0 + sp] >= 0:
                        aux[sp, VE0 + NB * sig + b] = 1e-7
                for tgt in (o, o + 1):
                    if s0 <= tgt < s0 + ps:
                        aux[tgt - s0, I00 + NB * sig + b] = 1.0
            tstar = int(ilen[b]) - 1
            cstar = tstar // TC
            jstar = tstar - cstar * TC + 1
            aux[32 * cstar + b, EM0 + jstar] = 1.0
            aux[32 * cstar + b, SEL0 + b] = 1.0
            aux[b, KB0] = -np.log(2.0) * 127.0 * (tstar + 1)
            aux[b, TM0:TM0 + tstar + 1] = 1.0

        in_maps.append({"big": big.astype(bf16), "aux": aux})
    return in_maps


def kernel(y_true, y_pred, input_lengths, label_lengths):
    y_true = np.asarray(y_true)
    y_pred = np.asarray(y_pred, dtype=np.float32)
    input_lengths = np.asarray(input_lengths)
    label_lengths = np.asarray(label_lengths)

    if "nc" not in _cached:
        _cached["nc"] = _build_program()
    nc = _cached["nc"]

    in_maps = _host_prep(y_true, y_pred, input_lengths, label_lengths)
    res = run_bass_kernel_spmd(nc, in_maps, core_ids=list(range(NCORE)))
    out = np.concatenate([res.results[i]["loss"] for i in range(NCORE)], axis=0)
    return out.astype(np.float32)


# revision 43
# speedup vs baseline: 1.0822x; 1.0822x over previous
"""CTC loss kernel for Trainium2 (Bass/Tile), 8-core data-parallel.

Per core (8 samples): linear-space CTC forward recurrence, scanned
column-by-column over the extended-label axis (S=201).  The time axis
(T=1000) lives on the free dim, split into 4 chunks of 250 mapped to the
four SBUF partition quadrants (partition = 32*chunk + sample).  Each
column is computed with a two-pass blocked scan: ONE full-width
zero-init tensor_tensor_scan covers all 4 chunks at once (partitions are
free parallelism), a second full-width scan forms the chunk prefix
products of q (with the cross-chunk scale conversion RB folded into its
initial value), a 3-step tiny carry recurrence links the chunk
boundaries, and one full-width fused multiply-add reconstructs
x = z + carry * P'.  Odd columns add one fused C-op (even columns have
skip==0 structurally, so the scans read the previous column's tile
directly).

Numerics: per-frame normalizer nu[t] = (1.2/(2l+1)) * sum_s y_pred[t,ext[s]]
(folded into the w matmul vector host-side) keeps drift to a random walk;
per-(sample,chunk) rescales every 16 columns (never scaling up, rho
exponent clamped, Ln computed with a 2^-32 prescale) keep everything in
f32; the final loss re-adds the log-nu prefix sum (N8) and the
accumulated log scales (lambda).  Matmul inputs and the q-hat DRAM
bounce are bf16 (the nu reciprocals are exact powers of two, so the
normalizer path stays exact); the scan state itself stays f32.

Host side does only label-index bookkeeping (one-hot gather matrices,
masks) plus a pure layout transpose of y_pred; all y_pred-dependent math
runs on device.
"""
import os
import sys

sys.path.insert(0, "/opt/trn_rl_repo")

import numpy as np

import concourse.bass as bass
import concourse.bacc as bacc
import concourse.mybir as mybir
import concourse.tile as tile
from concourse.bass_utils import run_bass_kernel_spmd

B, T, C, L = 64, 1000, 128, 100
S = 2 * L + 1            # 201
NB = 8                   # samples per core
NCORE = 8
NCH, TC = 4, 250         # time chunks x chunk length
RS = 16                  # rescale every RS columns
SBLK = 24                # columns per streamed block
KAPPA = 1.2              # normalizer constant (per-sample cK = KAPPA/(2l+1))
LN232 = 22.18070977791825   # 32*ln(2)
LN2 = 0.6931471805599453
EXPMASK = 0x7F800000
RCPBASE = 0x7F000000
F32 = mybir.dt.float32
BF16 = mybir.dt.bfloat16
INT32 = mybir.dt.int32
AOP = mybir.AluOpType
AFT = mybir.ActivationFunctionType

# big tensor column offsets (partition dim = C = 128), dtype bf16
YP0 = 0                  # ypT: col b*1000 + t
G0 = NB * T              # g:  col G0 + b*201 + s
W0 = G0 + NB * S         # w:  col W0 + b
SELB0 = W0 + NB          # sel8: 8 blocks of 128 (replication matmul weights)
WB0 = SELB0 + NB * 128   # wb: 8 blocks of 8 (w_b in column b, else zero)
BIGN = WB0 + NB * NB

# aux tensor column offsets (partition dim = 128), dtype f32
SKP0 = 0                 # skipm [128, S]
EM0 = SKP0 + S           # emask [128, TC+1]
VE0 = EM0 + TC + 1       # veps [128, 2*NB]
I00 = VE0 + 2 * NB       # ind0 [128, 2*NB]
SEL0 = I00 + 2 * NB      # sel [128, NB]
KB0 = SEL0 + NB          # per-sample N8 offset const [rows 0:8, 1]
TM0 = KB0 + 1            # tmask [rows 0:8, T]
AUXN = TM0 + T

_cached = {}


def _build_program():
    from contextlib import ExitStack

    nc = bacc.Bacc(None, target_bir_lowering=False)

    big_d = nc.dram_tensor("big", [C, BIGN], BF16, kind="ExternalInput")
    aux_d = nc.dram_tensor("aux", [128, AUXN], F32, kind="ExternalInput")
    loss_d = nc.dram_tensor("loss", [NB, 1], F32, kind="ExternalOutput")
    dbg_d = nc.dram_tensor("dbg", [4, 128], F32, kind="ExternalOutput")
    # internal bounce, split per sig so the scan's early reads only wait on
    # the sig0 writes
    qhat_ds = (nc.dram_tensor("qhat0", [NB, 128, T], BF16),
               nc.dram_tensor("qhat1", [NB, S - 128, T], BF16))

    PS_SIG = (128, S - 128)
    HB = 2                    # samples per gather PSUM round

    with tile.TileContext(nc) as tc, ExitStack() as ctx:
        pers = ctx.enter_context(tc.tile_pool(name="pers", bufs=1))
        pbig = ctx.enter_context(tc.tile_pool(name="pbig", bufs=2, space="PSUM"))
        psml = ctx.enter_context(tc.tile_pool(name="psml", bufs=2, space="PSUM"))
        qblk_pool = ctx.enter_context(tc.tile_pool(name="qblk", bufs=2))
        vrow = ctx.enter_context(tc.tile_pool(name="vrow", bufs=2))

        big = pers.tile([C, BIGN], BF16, tag="big")
        aux = pers.tile([128, AUXN], F32, tag="aux")
        n8parts = pers.tile([NB, 2], F32, tag="n8parts")
        rnusAll = pers.tile([NB, 2 * 500], BF16, tag="rnusAll")
        nurepS = pers.tile([128, 2, NB, 500], F32, tag="nurepS")
        stags = [pers.tile([128, NB, 500], BF16, tag=f"stag{i}",
                           name=f"stag{i}") for i in range(2)]
        junk = pers.tile([128, 501], F32, tag="junk")
        X = []
        for i in range(3):
            xt = pers.tile([128, TC + 1], F32, tag=f"X{i}", name=f"X{i}")
            X.append(xt)
        Cbuf = pers.tile([128, TC], F32, tag="Cbuf")
        sc = pers.tile([128, 8], F32, tag="sc")
        LAM, RHO, LSH, MRE, TMP, R199, R200, RTOT = range(8)
        sci = pers.tile([128, 8], INT32, tag="sci")
        LAMI, LSHI, DI, EI, RA, RB = range(6)
        lamF = pers.tile([128, 1], F32, tag="lamF")
        v8 = pers.tile([NB, 4], F32, tag="v8")
        N8c, LOGRc, T1c, LOSSc = range(4)

        def ypr(b, h):
            return big[:, YP0 + b * T + h * 500:YP0 + b * T + (h + 1) * 500]

        # ---- loads ----
        # weights/gather region first, then ypT in per-(b,h) slices so each
        # nu matmul starts as soon as its own slice lands
        nc.sync.dma_start(aux[:], aux_d[:])
        nc.sync.dma_start(big[:, G0:BIGN], big_d[:, G0:BIGN])
        for h in range(2):
            for b in range(NB):
                lo = YP0 + b * T + h * 500
                nc.sync.dma_start(big[:, lo:lo + 500], big_d[:, lo:lo + 500])

        # ---- nu phase (batched over the 8 samples) ----
        # accumulate all 8 samples' nu rows into one [8,500] PSUM tile per
        # half via one-hot-column weight blocks; the exponent chain then runs
        # batched on DVE straight out of PSUM.
        nui = vrow.tile([NB, 2 * 500], INT32, tag="nui_t")
        ef = vrow.tile([NB, 2 * 500], F32, tag="ef_t")
        tms = aux[0:NB, TM0:TM0 + T]
        for h in range(2):
            pnuh = psml.tile([NB, 512], F32, tag="psm")
            for b in range(NB):
                nc.tensor.matmul(pnuh[0:NB, 0:500],
                                 big[:, WB0 + b * NB:WB0 + (b + 1) * NB],
                                 ypr(b, h), start=(b == 0), stop=(b == NB - 1))
            # exponent bits of nu (power-of-two normalizer)
            nc.vector.tensor_scalar(nui[:, h * 500:(h + 1) * 500],
                                    pnuh[0:NB, 0:500].bitcast(INT32),
                                    EXPMASK, None, AOP.bitwise_and)
        # exact reciprocal: bits = RCPBASE - expbits
        nc.vector.tensor_scalar(ef.bitcast(INT32)[:], nui[:],
                                -1, RCPBASE, AOP.mult, AOP.add)
        # rnus (bf16, exact powers of two) masked by tmask
        nc.vector.tensor_tensor(rnusAll[:], ef[:], tms, AOP.mult)
        # E field as f32 for the N8 sum
        nc.vector.tensor_scalar(nui[:], nui[:],
                                23, None, AOP.logical_shift_right)
        nc.vector.tensor_copy(ef[:], nui[:])
        nc.vector.scalar_tensor_tensor(
            junk[0:NB, 0:500], ef[:, 0:500], 1.0, tms[:, 0:500],
            AOP.mult, AOP.mult, accum_out=n8parts[:, 0:1])
        nc.vector.scalar_tensor_tensor(
            junk[0:NB, 0:500], ef[:, 500:1000], 1.0, tms[:, 500:1000],
            AOP.mult, AOP.mult, accum_out=n8parts[:, 1:2])

        # rnus replication across all 128 partitions happens inline in the
        # sig0 gather below, so the first gather STT is not gated by all 16
        # replication matmuls
        def emit_repl(b, h):
            prep = psml.tile([128, 512], F32, tag="prep")
            nc.tensor.matmul(prep[:, 0:500],
                             big[0:NB, SELB0 + b * 128:SELB0 + (b + 1) * 128],
                             rnusAll[:, h * 500:(h + 1) * 500],
                             start=True, stop=True)
            nc.scalar.copy(nurepS[:, h, b, :], prep[:, 0:500])

        # ---- gather phases ----
        def emit_gather(sig):
            ps = PS_SIG[sig]
            s0 = 0 if sig == 0 else 128
            for h in range(2):
                for hb in range(NB // HB):
                    if sig == 0:
                        for bb in range(HB):
                            emit_repl(hb * HB + bb, h)
                    gat = pbig.tile([128, HB, 512], F32, tag="gat")
                    for bb in range(HB):
                        b = hb * HB + bb
                        nc.tensor.matmul(
                            gat[0:ps, bb, 0:500],
                            big[:, G0 + b * S + s0:G0 + b * S + s0 + ps],
                            ypr(b, h), start=True, stop=True)
                    for bb in range(HB):
                        b = hb * HB + bb
                        nc.vector.scalar_tensor_tensor(
                            stags[h][0:ps, b, :], gat[0:ps, bb, 0:500],
                            aux[0:ps, VE0 + NB * sig + b:VE0 + NB * sig + b + 1],
                            nurepS[0:ps, h, b, :],
                            AOP.add, AOP.mult)
                if h == 0:
                    nc.vector.tensor_tensor(
                        stags[0][0:ps, :, 0], stags[0][0:ps, :, 0],
                        aux[0:ps, I00 + NB * sig:I00 + NB * (sig + 1)],
                        AOP.mult)
                nc.sync.dma_start(
                    qhat_ds[sig][:, :, h * 500:(h + 1) * 500]
                    .rearrange("b s j -> s b j"),
                    stags[h][0:ps, :, :])

        emit_gather(0)

        # preload the Ln activation table while the Activation engine is
        # idle, so the finalize's Ln does not pay the table load
        actwarm = pers.tile([1, 1], F32, tag="actwarm")
        nc.gpsimd.memset(actwarm[:], 1.0)
        nc.scalar.activation(actwarm[:], actwarm[:], AFT.Ln)

        # ---- scan phase (two-pass per column) ----
        # Pass 1: ONE full-width zero-init scan covers all 4 time chunks
        # (partitions are free parallelism).  Pass 2: x = z + carry * P'
        # where P' is the chunk prefix-product of q with the carry scale
        # conversion RB folded into the scan's initial value, and the
        # carries follow a tiny 3-step recurrence.  This replaces 4 chained
        # 250-long scans with 2 full-width scans + 1 full-width STT.
        zb = pers.tile([128, TC + 1], F32, tag="zb")
        Pb = [pers.tile([128, TC + 1], F32, tag=f"Pb{i}", name=f"Pb{i}")
              for i in range(2)]
        ones250 = pers.tile([128, TC], F32, tag="ones250")
        cvec = pers.tile([128, 1], F32, tag="cvec")
        rb01 = pers.tile([128, 1], F32, tag="rb01")
        RCPI = 6              # sci scratch column for the rescale reciprocal
        for i in range(3):
            nc.gpsimd.memset(X[i][:], 0.0)
            nc.gpsimd.memset(X[i][0:NB, 0:1], 1.0)
        nc.gpsimd.memset(ones250[:], 1.0)
        nc.gpsimd.memset(zb[:, 0:1], 0.0)
        nc.gpsimd.memset(sc[:], 0.0)
        nc.gpsimd.memset(sci[:], 0)
        nc.gpsimd.memset(sci[:, RB:RB + 1], 0x3F800000)
        # chunk 0 has no incoming carry: P'[0:NB] = 0 makes the full-width
        # fixup a no-op there; its cvec slot holds the 1.0 virtual-init seed
        nc.gpsimd.memset(sci[0:NB, RB:RB + 1], 0)
        nc.gpsimd.memset(cvec[:], 0.0)
        nc.gpsimd.memset(cvec[0:NB, 0:1], 1.0)
        nc.gpsimd.memset(rb01[:], 1.0)

        sblocks = []
        s = 0
        while s < S:
            n = min(SBLK, S - s)
            if s < 128 < s + n:
                n = 128 - s          # align a block boundary at the sig split
            if S - (s + n) == 1:
                n += 1
            sblocks.append((s, n))
            s += n
        col2bi = {}
        for bi, (sb, nsb) in enumerate(sblocks):
            for k in range(nsb):
                col2bi[sb + k] = bi

        rbF = sci.bitcast(F32)
        plam8_holder = []
        # two persistent ping-pong q tiles; the full-width scans read every
        # partition, so zero the unused lanes once up front (the DMAs only
        # ever write the real lanes, so they stay zero)
        max_nsb = max(n for _, n in sblocks)
        qtiles = [pers.tile([128, max_nsb, TC], BF16, tag=f"qt{i}",
                            name=f"qt{i}") for i in range(2)]
        for qt in qtiles:
            nc.gpsimd.memset(qt[:], 0.0)

        def load_block(bi):
            sb, nsb = sblocks[bi]
            qblk = qtiles[bi % 2]
            sig = 0 if sb < 128 else 1
            qsrc = qhat_ds[sig]
            qs0 = sb - (0 if sig == 0 else 128)
            for c in range(NCH):
                nc.sync.dma_start(
                    qblk[32 * c:32 * c + NB, 0:nsb, :],
                    qsrc[:, qs0:qs0 + nsb, c * TC:(c + 1) * TC])

        def qop(s):
            bi = col2bi[s]
            return qtiles[bi % 2], s - sblocks[bi][0]

        def emit_P(s):
            qb, kk = qop(s)
            nc.vector.tensor_tensor_scan(
                Pb[s % 2][:, 1:TC + 1], ones250[:], qb[:, kk, :],
                rbF[:, RB:RB + 1], AOP.mult, AOP.mult)

        load_block(0)
        emit_P(0)
        for bi, (sb, nsb) in enumerate(sblocks):
            if bi == 1:
                # sig1's gather rides behind the first block's scan columns:
                # its data is only needed ~5 blocks later
                emit_gather(1)
            if bi + 1 < len(sblocks):
                load_block(bi + 1)      # one block of DMA lookahead
            for k in range(nsb):
                s = sb + k
                xs = X[s % 3]
                xm1 = X[(s + 2) % 3]
                xm2 = X[(s + 1) % 3]
                qb, kk = qop(s)
                Pcur = Pb[s % 2]
                if s % 2 == 1:
                    # odd columns: C = xm1 + skip*xm2 (skip can be nonzero)
                    nc.vector.scalar_tensor_tensor(
                        Cbuf[:], xm2[:, 0:TC], aux[:, SKP0 + s:SKP0 + s + 1],
                        xm1[:, 0:TC], AOP.mult, AOP.add)
                    d0 = Cbuf
                else:
                    # even columns are blanks: skip == 0 for every sample, so
                    # C = xm1 and the scan reads the previous column directly
                    d0 = xm1
                # pass 1: zero-init scans of all 4 chunks, one instruction
                nc.vector.tensor_tensor_scan(
                    zb[:, 1:TC + 1], d0[:, 0:TC], qb[:, kk, :],
                    0.0, AOP.add, AOP.mult)
                epoch = (s + 1) % RS == 0 and s < 198
                if s + 1 < S and not epoch:
                    emit_P(s + 1)       # filler: hides carry-chain latency
                # tiny carry recurrence on RAW (unconverted) carries:
                # c[n+1] = c[n]*P'(end) + z(end) = x[n](end) in frame n
                for c in range(NCH - 1):
                    lo = 32 * c
                    nc.vector.scalar_tensor_tensor(
                        cvec[lo + 32:lo + 40, 0:1],
                        Pcur[lo:lo + NB, TC:TC + 1],
                        cvec[lo:lo + NB, 0:1],
                        zb[lo:lo + NB, TC:TC + 1],
                        AOP.mult, AOP.add)
                # chunk-boundary values for the next columns' C reads, in
                # each destination chunk's scale frame (seed lanes stay 1.0)
                nc.vector.tensor_tensor(xs[:, 0:1], cvec[:], rb01[:],
                                        AOP.mult)
                # pass 2 fixup: x = z + carry * P'
                nc.vector.scalar_tensor_tensor(
                    xs[:, 1:TC + 1], Pcur[:, 1:TC + 1], cvec[:, 0:1],
                    zb[:, 1:TC + 1], AOP.mult, AOP.add)
                if s in (199, 200):
                    rcol = R199 if s == 199 else R200
                    nc.vector.scalar_tensor_tensor(
                        junk[:, 0:TC + 1], xs[:], 1.0,
                        aux[:, EM0:EM0 + TC + 1],
                        AOP.mult, AOP.mult, accum_out=sc[:, rcol:rcol + 1])
                if s == 193:
                    # lambda is final after the last epoch (col 191): move
                    # its readout matmul off the serial finalize tail
                    nc.vector.tensor_copy(lamF[:], sci[:, LAMI:LAMI + 1])
                    nc.vector.tensor_scalar_mul(lamF[:], lamF[:], LN2)
                    plam8 = psml.tile([NB, 512], F32, tag="prep")
                    nc.tensor.matmul(plam8[:, 0:1], aux[:, SEL0:SEL0 + NB],
                                     lamF[:], start=True, stop=True)
                    plam8_holder.append(plam8)
                if epoch:
                    nc.vector.tensor_reduce(
                        sc[:, MRE:MRE + 1], xs[:], mybir.AxisListType.X,
                        AOP.max, apply_absolute_value=True)
                    nc.vector.tensor_scalar_max(
                        sc[:, MRE:MRE + 1], sc[:, MRE:MRE + 1], 1.0)
                    # exponent-bit games: exact power-of-two rescale
                    nc.vector.tensor_scalar(
                        sci[:, RA:RA + 1], sc[:, MRE:MRE + 1].bitcast(INT32),
                        EXPMASK, None, AOP.bitwise_and)
                    nc.vector.tensor_scalar(
                        sci[:, RCPI:RCPI + 1], sci[:, RA:RA + 1],
                        -1, RCPBASE, AOP.mult, AOP.add)
                    rcpf = sci.bitcast(F32)[:, RCPI:RCPI + 1]
                    nc.vector.tensor_scalar_mul(xs[:], xs[:], rcpf)
                    nc.vector.tensor_scalar_mul(xm1[:], xm1[:], rcpf)
                    # restore the virtual-init seeds (DVE, not Pool memset,
                    # to avoid two cross-engine round trips mid-epoch)
                    nc.vector.tensor_scalar(xs[0:NB, 0:1], xs[0:NB, 0:1],
                                            0.0, 1.0, AOP.mult, AOP.add)
                    nc.vector.tensor_scalar(xm1[0:NB, 0:1], xm1[0:NB, 0:1],
                                            0.0, 1.0, AOP.mult, AOP.add)
                    nc.vector.tensor_scalar(
                        sci[:, EI:EI + 1], sci[:, RA:RA + 1],
                        23, None, AOP.logical_shift_right)
                    nc.vector.tensor_scalar(
                        sci[:, EI:EI + 1], sci[:, EI:EI + 1],
                        127, None, AOP.subtract)
                    nc.vector.tensor_tensor(sci[:, LAMI:LAMI + 1],
                                            sci[:, LAMI:LAMI + 1],
                                            sci[:, EI:EI + 1], AOP.add)
                    nc.vector.tensor_copy(sci[32:64, LSHI:LSHI + 1],
                                          sci[0:32, LAMI:LAMI + 1])
                    nc.vector.tensor_copy(sci[64:96, LSHI:LSHI + 1],
                                          sci[32:64, LAMI:LAMI + 1])
                    nc.vector.tensor_copy(sci[96:128, LSHI:LSHI + 1],
                                          sci[64:96, LAMI:LAMI + 1])
                    nc.vector.tensor_tensor(sci[:, DI:DI + 1],
                                            sci[:, LSHI:LSHI + 1],
                                            sci[:, LAMI:LAMI + 1],
                                            AOP.subtract)
                    nc.vector.tensor_scalar(sci[:, DI:DI + 1],
                                            sci[:, DI:DI + 1],
                                            126, -126, AOP.min, AOP.max)
                    nc.vector.tensor_scalar(sci[:, RB:RB + 1],
                                            sci[:, DI:DI + 1],
                                            127, None, AOP.add)
                    nc.vector.tensor_scalar(sci[:, RB:RB + 1],
                                            sci[:, RB:RB + 1],
                                            23, None, AOP.logical_shift_left)
                    # chunk-0 lanes must keep P' == 0 (no incoming carry)
                    nc.vector.tensor_scalar(sci[0:NB, RB:RB + 1],
                                            sci[0:NB, RB:RB + 1],
                                            0, None, AOP.mult)
                    # boundary-conversion vector: RB everywhere except the
                    # chunk-0 seed lanes, which stay exactly 1.0
                    nc.vector.tensor_copy(rb01[:], rbF[:, RB:RB + 1])
                    nc.vector.tensor_scalar(rb01[0:NB, 0:1], rb01[0:NB, 0:1],
                                            0.0, 1.0, AOP.mult, AOP.add)
                    emit_P(s + 1)       # uses the refreshed RB

        # ---- finalize ----
        nc.vector.tensor_tensor(sc[:, RTOT:RTOT + 1], sc[:, R199:R199 + 1],
                                sc[:, R200:R200 + 1], AOP.add)
        pr8 = psml.tile([NB, 512], F32, tag="psm")
        nc.tensor.matmul(pr8[:, 0:1], aux[:, SEL0:SEL0 + NB],
                         sc[:, RTOT:RTOT + 1], start=True, stop=True)
        plam8 = plam8_holder[0]
        nc.vector.tensor_tensor(v8[:, N8c:N8c + 1], n8parts[:, 0:1],
                                n8parts[:, 1:2], AOP.add)
        nc.vector.scalar_tensor_tensor(
            v8[:, N8c:N8c + 1], v8[:, N8c:N8c + 1], LN2,
            aux[0:NB, KB0:KB0 + 1], AOP.mult, AOP.add)
        # split r = m * 2^(E-127), m in [1,2): exact exponent, Ln on mantissa
        ri8 = pers.tile([NB, 2], INT32, tag="ri8")
        rf8 = pers.tile([NB, 2], F32, tag="rf8")
        nc.vector.tensor_scalar(ri8[:, 0:1], pr8[:, 0:1].bitcast(INT32),
                                23, None, AOP.logical_shift_right)
        nc.vector.tensor_copy(rf8[:, 0:1], ri8[:, 0:1])
        nc.vector.tensor_scalar(ri8[:, 1:2], pr8[:, 0:1].bitcast(INT32),
                                0x007FFFFF, 0x3F800000,
                                AOP.bitwise_and, AOP.bitwise_or)
        nc.scalar.activation(v8[:, LOGRc:LOGRc + 1],
                             ri8.bitcast(F32)[:, 1:2], AFT.Ln)
        nc.vector.tensor_scalar(rf8[:, 0:1], rf8[:, 0:1],
                                127.0, LN2, AOP.subtract, AOP.mult)
        nc.vector.tensor_tensor(v8[:, LOGRc:LOGRc + 1],
                                v8[:, LOGRc:LOGRc + 1],
                                rf8[:, 0:1], AOP.add)
        nc.vector.tensor_tensor(v8[:, T1c:T1c + 1], v8[:, LOGRc:LOGRc + 1],
                                v8[:, N8c:N8c + 1], AOP.add)
        nc.vector.scalar_tensor_tensor(
            v8[:, LOSSc:LOSSc + 1], v8[:, T1c:T1c + 1], -1.0, plam8[:, 0:1],
            AOP.mult, AOP.subtract)
        nc.sync.dma_start(loss_d[:], v8[:, LOSSc:LOSSc + 1])

    nc.finalize()
    return nc


def _host_prep(y_true, y_pred, input_lengths, label_lengths):
    bf16 = mybir.dt.np(BF16)
    in_maps = []
    for core in range(NCORE):
        bsl = slice(core * NB, (core + 1) * NB)
        yt = y_true[bsl]
        ilen = input_lengths[bsl].astype(np.int64)
        llen = label_lengths[bsl].astype(np.int64)

        big = np.zeros((C, BIGN), np.float32)
        big[:, YP0:YP0 + NB * T] = (
            y_pred[bsl].transpose(2, 0, 1).reshape(C, NB * T))
        aux = np.zeros((128, AUXN), np.float32)

        for b in range(NB):
            l = int(llen[b]); o = 200 - 2 * l
            ext = np.full(S, -1, np.int32)
            for k in range(2 * l + 1):
                ext[o + k] = C - 1 if k % 2 == 0 else yt[b, (k - 1) // 2]
            gb = np.zeros((C, S), np.float32)
            for s in range(S):
                if ext[s] >= 0:
                    gb[ext[s], s] = 1.0
                k = s - o
                if k >= 2 and k % 2 == 1 and ext[s] != ext[s - 2]:
                    for c in range(NCH):
                        aux[32 * c + b, SKP0 + s] = 1.0
            big[:, G0 + b * S:G0 + (b + 1) * S] = gb
            wvec = gb.sum(axis=1) * np.float32(KAPPA * np.sqrt(2.0) / (2 * l + 1))
            big[:, W0 + b] = wvec
            big[b, SELB0 + b * 128:SELB0 + (b + 1) * 128] = 1.0
            big[:, WB0 + b * NB + b] = wvec
            for sig in range(2):
                s0, ps = (0, 128) if sig == 0 else (128, S - 128)
                for sp in range(ps):
                    if ext[s0 + sp] >= 0:
                        aux[sp, VE0 + NB * sig + b] = 1e-7
                for tgt in (o, o + 1):
                    if s0 <= tgt < s0 + ps:
                        aux[tgt - s0, I00 + NB * sig + b] = 1.0
            tstar = int(ilen[b]) - 1
            cstar = tstar // TC
            jstar = tstar - cstar * TC + 1
            aux[32 * cstar + b, EM0 + jstar] = 1.0
            aux[32 * cstar + b, SEL0 + b] = 1.0
            aux[b, KB0] = -np.log(2.0) * 127.0 * (tstar + 1)
            aux[b, TM0:TM0 + tstar + 1] = 1.0

        in_maps.append({"big": big.astype(bf16), "aux": aux})
    return in_maps


def kernel(y_true, y_pred, input_lengths, label_lengths):
    y_true = np.asarray(y_true)
    y_pred = np.asarray(y_pred, dtype=np.float32)
    input_lengths = np.asarray(input_lengths)
    label_lengths = np.asarray(label_lengths)

    if "nc" not in _cached:
        _cached["nc"] = _build_program()
    nc = _cached["nc"]

    in_maps = _host_prep(y_true, y_pred, input_lengths, label_lengths)
    res = run_bass_kernel_spmd(nc, in_maps, core_ids=list(range(NCORE)))
    out = np.concatenate([res.results[i]["loss"] for i in range(NCORE)], axis=0)
    return out.astype(np.float32)


# revision 51
# speedup vs baseline: 1.1015x; 1.0179x over previous
"""CTC loss kernel for Trainium2 (Bass/Tile), 8-core data-parallel.

Per core (8 samples): linear-space CTC forward recurrence, scanned
column-by-column over the extended-label axis (S=201).  The time axis
(T=1000) lives on the free dim, split into 4 chunks of 250 mapped to the
four SBUF partition quadrants (partition = 32*chunk + sample).  Each
column is computed with a two-pass blocked scan: ONE full-width
zero-init tensor_tensor_scan covers all 4 chunks at once (partitions are
free parallelism), a second full-width scan forms the chunk prefix
products of q (with the cross-chunk scale conversion RB folded into its
initial value), a 3-step tiny carry recurrence links the chunk
boundaries, and one full-width fused multiply-add reconstructs
x = z + carry * P'.  Odd columns add one fused C-op (even columns have
skip==0 structurally, so the scans read the previous column's tile
directly).

Numerics: per-frame normalizer nu[t] = (1.2/(2l+1)) * sum_s y_pred[t,ext[s]]
(folded into the w matmul vector host-side) keeps drift to a random walk;
per-(sample,chunk) rescales every 16 columns (never scaling up, rho
exponent clamped, Ln computed with a 2^-32 prescale) keep everything in
f32; the final loss re-adds the log-nu prefix sum (N8) and the
accumulated log scales (lambda).  Matmul inputs and the q-hat DRAM
bounce are bf16 (the nu reciprocals are exact powers of two, so the
normalizer path stays exact); the scan state itself stays f32.

Host side does only label-index bookkeeping (one-hot gather matrices,
masks) plus a pure layout transpose of y_pred; all y_pred-dependent math
runs on device.
"""
import os
import sys

sys.path.insert(0, "/opt/trn_rl_repo")

import numpy as np

import concourse.bass as bass
import concourse.bacc as bacc
import concourse.mybir as mybir
import concourse.tile as tile
from concourse.bass_utils import run_bass_kernel_spmd

B, T, C, L = 64, 1000, 128, 100
S = 2 * L + 1            # 201
NB = 8                   # samples per core
NCORE = 8
NCH, TC = 4, 250         # time chunks x chunk length
RS = 16                  # rescale every RS columns
SBLK = 24                # columns per streamed block
KAPPA = 1.2              # normalizer constant (per-sample cK = KAPPA/(2l+1))
LN232 = 22.18070977791825   # 32*ln(2)
LN2 = 0.6931471805599453
EXPMASK = 0x7F800000
RCPBASE = 0x7F000000
F32 = mybir.dt.float32
BF16 = mybir.dt.bfloat16
INT32 = mybir.dt.int32
AOP = mybir.AluOpType
AFT = mybir.ActivationFunctionType

# big tensor column offsets (partition dim = C = 128), dtype bf16
YP0 = 0                  # ypT: col b*1000 + t
G0 = NB * T              # g:  col G0 + b*201 + s
W0 = G0 + NB * S         # w:  col W0 + b
SELB0 = W0 + NB          # sel8: 8 blocks of 128 (replication matmul weights)
WB0 = SELB0 + NB * 128   # wb: 8 blocks of 8 (w_b in column b, else zero)
BIGN = WB0 + NB * NB

# aux tensor column offsets (partition dim = 128), dtype f32
SKP0 = 0                 # skipm [128, S]
EM0 = SKP0 + S           # emask [128, TC+1]
VE0 = EM0 + TC + 1       # veps [128, 2*NB]
I00 = VE0 + 2 * NB       # ind0 [128, 2*NB]
SEL0 = I00 + 2 * NB      # sel [128, NB]
KB0 = SEL0 + NB          # per-sample N8 offset const [rows 0:8, 1]
TM0 = KB0 + 1            # tmask [rows 0:8, T]
AUXN = TM0 + T

_cached = {}


def _build_program():
    from contextlib import ExitStack

    nc = bacc.Bacc(None, target_bir_lowering=False)

    big_d = nc.dram_tensor("big", [C, BIGN], BF16, kind="ExternalInput")
    aux_d = nc.dram_tensor("aux", [128, AUXN], F32, kind="ExternalInput")
    loss_d = nc.dram_tensor("loss", [NB, 1], F32, kind="ExternalOutput")
    dbg_d = nc.dram_tensor("dbg", [4, 128], F32, kind="ExternalOutput")
    # internal bounce, split per sig so the scan's early reads only wait on
    # the sig0 writes
    qhat_ds = (nc.dram_tensor("qhat0", [NB, 128, T], BF16),
               nc.dram_tensor("qhat1", [NB, S - 128, T], BF16))

    PS_SIG = (128, S - 128)
    HB = 2                    # samples per gather PSUM round

    with tile.TileContext(nc) as tc, ExitStack() as ctx:
        pers = ctx.enter_context(tc.tile_pool(name="pers", bufs=1))
        pbig = ctx.enter_context(tc.tile_pool(name="pbig", bufs=2, space="PSUM"))
        psml = ctx.enter_context(tc.tile_pool(name="psml", bufs=2, space="PSUM"))
        qblk_pool = ctx.enter_context(tc.tile_pool(name="qblk", bufs=2))
        vrow = ctx.enter_context(tc.tile_pool(name="vrow", bufs=2))

        big = pers.tile([C, BIGN], BF16, tag="big")
        aux = pers.tile([128, AUXN], F32, tag="aux")
        n8parts = pers.tile([NB, 2], F32, tag="n8parts")
        rnusAll = pers.tile([NB, 2 * 500], BF16, tag="rnusAll")
        nurepS = pers.tile([128, 2, NB, 500], F32, tag="nurepS")
        stags = [pers.tile([128, NB, 500], BF16, tag=f"stag{i}",
                           name=f"stag{i}") for i in range(2)]
        junk = pers.tile([128, 501], F32, tag="junk")
        X = []
        for i in range(3):
            xt = pers.tile([128, TC + 1], F32, tag=f"X{i}", name=f"X{i}")
            X.append(xt)
        Cbuf = pers.tile([128, TC], F32, tag="Cbuf")
        sc = pers.tile([128, 8], F32, tag="sc")
        LAM, RHO, LSH, MRE, TMP, R199, R200, RTOT = range(8)
        sci = pers.tile([128, 8], INT32, tag="sci")
        LAMI, LSHI, DI, EI, RA, RB = range(6)
        lamF = pers.tile([128, 1], F32, tag="lamF")
        v8 = pers.tile([NB, 4], F32, tag="v8")
        N8c, LOGRc, T1c, LOSSc = range(4)

        def ypr(b, h):
            return big[:, YP0 + b * T + h * 500:YP0 + b * T + (h + 1) * 500]

        # ---- loads ----
        # DMA queue order mirrors consumption order: the tiny tmask rows,
        # then weights/gather region, then ypT h0 slices (nu + gather h0),
        # then the rest of aux (veps/masks, needed by the first STT), then
        # ypT h1
        nc.sync.dma_start(aux[0:NB, TM0:TM0 + T], aux_d[0:NB, TM0:TM0 + T])
        nc.sync.dma_start(big[:, G0:BIGN], big_d[:, G0:BIGN])
        for h in range(2):
            for b in range(NB):
                lo = YP0 + b * T + h * 500
                nc.sync.dma_start(big[:, lo:lo + 500], big_d[:, lo:lo + 500])
            if h == 0:
                nc.sync.dma_start(aux[:, 0:TM0], aux_d[:, 0:TM0])

        # ---- nu phase (batched over the 8 samples) ----
        # accumulate all 8 samples' nu rows into one [8,500] PSUM tile per
        # half via one-hot-column weight blocks; the exponent chain then runs
        # batched on DVE straight out of PSUM.
        nui = vrow.tile([NB, 2 * 500], INT32, tag="nui_t")
        ef = vrow.tile([NB, 2 * 500], F32, tag="ef_t")
        tms = aux[0:NB, TM0:TM0 + T]
        for h in range(2):
            hsl = slice(h * 500, (h + 1) * 500)
            pnuh = psml.tile([NB, 512], F32, tag="psm")
            for b in range(NB):
                nc.tensor.matmul(pnuh[0:NB, 0:500],
                                 big[:, WB0 + b * NB:WB0 + (b + 1) * NB],
                                 ypr(b, h), start=(b == 0), stop=(b == NB - 1))
            # per-half exponent chain so h0's rnus unblocks the gather while
            # h1's nu matmuls are still running
            nc.vector.tensor_scalar(nui[:, hsl],
                                    pnuh[0:NB, 0:500].bitcast(INT32),
                                    EXPMASK, None, AOP.bitwise_and)
            # exact reciprocal: bits = RCPBASE - expbits
            nc.vector.tensor_scalar(ef.bitcast(INT32)[:, hsl], nui[:, hsl],
                                    -1, RCPBASE, AOP.mult, AOP.add)
            # rnus (bf16, exact powers of two) masked by tmask
            nc.vector.tensor_tensor(rnusAll[:, hsl], ef[:, hsl],
                                    tms[:, hsl], AOP.mult)

        def emit_n8():
            # E-field sum for the loss reconstruction; not on the critical
            # path, rides behind the first scan block
            nc.vector.tensor_scalar(nui[:], nui[:],
                                    23, None, AOP.logical_shift_right)
            nc.vector.tensor_copy(ef[:], nui[:])
            nc.vector.scalar_tensor_tensor(
                junk[0:NB, 0:500], ef[:, 0:500], 1.0, tms[:, 0:500],
                AOP.mult, AOP.mult, accum_out=n8parts[:, 0:1])
            nc.vector.scalar_tensor_tensor(
                junk[0:NB, 0:500], ef[:, 500:1000], 1.0, tms[:, 500:1000],
                AOP.mult, AOP.mult, accum_out=n8parts[:, 1:2])

        # rnus replication across all 128 partitions happens inline in the
        # sig0 gather below, so the first gather STT is not gated by all 16
        # replication matmuls
        def emit_repl(b, h):
            prep = psml.tile([128, 512], F32, tag="prep")
            nc.tensor.matmul(prep[:, 0:500],
                             big[0:NB, SELB0 + b * 128:SELB0 + (b + 1) * 128],
                             rnusAll[:, h * 500:(h + 1) * 500],
                             start=True, stop=True)
            nc.scalar.copy(nurepS[:, h, b, :], prep[:, 0:500])

        # ---- gather phases ----
        def emit_gather(sig):
            ps = PS_SIG[sig]
            s0 = 0 if sig == 0 else 128
            for h in range(2):
                for hb in range(NB // HB):
                    if sig == 0:
                        for bb in range(HB):
                            emit_repl(hb * HB + bb, h)
                    gat = pbig.tile([128, HB, 512], F32, tag="gat")
                    for bb in range(HB):
                        b = hb * HB + bb
                        nc.tensor.matmul(
                            gat[0:ps, bb, 0:500],
                            big[:, G0 + b * S + s0:G0 + b * S + s0 + ps],
                            ypr(b, h), start=True, stop=True)
                    for bb in range(HB):
                        b = hb * HB + bb
                        nc.vector.scalar_tensor_tensor(
                            stags[h][0:ps, b, :], gat[0:ps, bb, 0:500],
                            aux[0:ps, VE0 + NB * sig + b:VE0 + NB * sig + b + 1],
                            nurepS[0:ps, h, b, :],
                            AOP.add, AOP.mult)
                if h == 0:
                    nc.vector.tensor_tensor(
                        stags[0][0:ps, :, 0], stags[0][0:ps, :, 0],
                        aux[0:ps, I00 + NB * sig:I00 + NB * (sig + 1)],
                        AOP.mult)
                nc.sync.dma_start(
                    qhat_ds[sig][:, :, h * 500:(h + 1) * 500]
                    .rearrange("b s j -> s b j"),
                    stags[h][0:ps, :, :])

        emit_gather(0)

        # preload the Ln activation table while the Activation engine is
        # idle, so the finalize's Ln does not pay the table load
        actwarm = pers.tile([1, 1], F32, tag="actwarm")
        nc.gpsimd.memset(actwarm[:], 1.0)
        nc.scalar.activation(actwarm[:], actwarm[:], AFT.Ln)

        # ---- scan phase (two-pass per column) ----
        # Pass 1: ONE full-width zero-init scan covers all 4 time chunks
        # (partitions are free parallelism).  Pass 2: x = z + carry * P'
        # where P' is the chunk prefix-product of q with the carry scale
        # conversion RB folded into the scan's initial value, and the
        # carries follow a tiny 3-step recurrence.  This replaces 4 chained
        # 250-long scans with 2 full-width scans + 1 full-width STT.
        zb = pers.tile([128, TC + 1], F32, tag="zb")
        Pb = [pers.tile([128, TC + 1], F32, tag=f"Pb{i}", name=f"Pb{i}")
              for i in range(2)]
        ones250 = pers.tile([128, TC], F32, tag="ones250")
        cvec = pers.tile([128, 1], F32, tag="cvec")
        rb01 = pers.tile([128, 1], F32, tag="rb01")
        RCPI = 6              # sci scratch column for the rescale reciprocal
        for i in range(3):
            nc.gpsimd.memset(X[i][:], 0.0)
            nc.gpsimd.memset(X[i][0:NB, 0:1], 1.0)
        nc.gpsimd.memset(ones250[:], 1.0)
        nc.gpsimd.memset(zb[:, 0:1], 0.0)
        nc.gpsimd.memset(sc[:], 0.0)
        nc.gpsimd.memset(sci[:], 0)
        nc.gpsimd.memset(sci[:, RB:RB + 1], 0x3F800000)
        # chunk 0 has no incoming carry: P'[0:NB] = 0 makes the full-width
        # fixup a no-op there; its cvec slot holds the 1.0 virtual-init seed
        nc.gpsimd.memset(sci[0:NB, RB:RB + 1], 0)
        nc.gpsimd.memset(cvec[:], 0.0)
        nc.gpsimd.memset(cvec[0:NB, 0:1], 1.0)
        nc.gpsimd.memset(rb01[:], 1.0)

        sblocks = []
        s = 0
        while s < S:
            n = min(SBLK, S - s)
            if s < 128 < s + n:
                n = 128 - s          # align a block boundary at the sig split
            if S - (s + n) == 1:
                n += 1
            sblocks.append((s, n))
            s += n
        col2bi = {}
        for bi, (sb, nsb) in enumerate(sblocks):
            for k in range(nsb):
                col2bi[sb + k] = bi

        rbF = sci.bitcast(F32)
        plam8_holder = []
        # two persistent ping-pong q tiles; the full-width scans read every
        # partition, so zero the unused lanes once up front (the DMAs only
        # ever write the real lanes, so they stay zero)
        max_nsb = max(n for _, n in sblocks)
        qtiles = [pers.tile([128, max_nsb, TC], BF16, tag=f"qt{i}",
                            name=f"qt{i}") for i in range(2)]
        for qt in qtiles:
            nc.gpsimd.memset(qt[:], 0.0)

        def load_block(bi):
            sb, nsb = sblocks[bi]
            qblk = qtiles[bi % 2]
            sig = 0 if sb < 128 else 1
            qsrc = qhat_ds[sig]
            qs0 = sb - (0 if sig == 0 else 128)
            for c in range(NCH):
                nc.sync.dma_start(
                    qblk[32 * c:32 * c + NB, 0:nsb, :],
                    qsrc[:, qs0:qs0 + nsb, c * TC:(c + 1) * TC])

        def qop(s):
            bi = col2bi[s]
            return qtiles[bi % 2], s - sblocks[bi][0]

        def emit_P(s):
            qb, kk = qop(s)
            nc.vector.tensor_tensor_scan(
                Pb[s % 2][:, 1:TC + 1], ones250[:], qb[:, kk, :],
                rbF[:, RB:RB + 1], AOP.mult, AOP.mult)

        load_block(0)
        emit_P(0)
        for bi, (sb, nsb) in enumerate(sblocks):
            if bi == 1:
                # sig1's gather and the N8 sums ride behind the first
                # block's scan columns: needed much later
                emit_gather(1)
                emit_n8()
            if bi + 1 < len(sblocks):
                load_block(bi + 1)      # one block of DMA lookahead
            for k in range(nsb):
                s = sb + k
                xs = X[s % 3]
                xm1 = X[(s + 2) % 3]
                xm2 = X[(s + 1) % 3]
                qb, kk = qop(s)
                Pcur = Pb[s % 2]
                if s % 2 == 1:
                    # odd columns: C = xm1 + skip*xm2 (skip can be nonzero)
                    nc.vector.scalar_tensor_tensor(
                        Cbuf[:], xm2[:, 0:TC], aux[:, SKP0 + s:SKP0 + s + 1],
                        xm1[:, 0:TC], AOP.mult, AOP.add)
                    d0 = Cbuf
                else:
                    # even columns are blanks: skip == 0 for every sample, so
                    # C = xm1 and the scan reads the previous column directly
                    d0 = xm1
                # pass 1: zero-init scans of all 4 chunks, one instruction
                nc.vector.tensor_tensor_scan(
                    zb[:, 1:TC + 1], d0[:, 0:TC], qb[:, kk, :],
                    0.0, AOP.add, AOP.mult)
                epoch = (s + 1) % RS == 0 and s < 198
                if s + 1 < S and not epoch:
                    emit_P(s + 1)       # filler: hides carry-chain latency
                # tiny carry recurrence on RAW (unconverted) carries:
                # c[n+1] = c[n]*P'(end) + z(end) = x[n](end) in frame n
                for c in range(NCH - 1):
                    lo = 32 * c
                    nc.vector.scalar_tensor_tensor(
                        cvec[lo + 32:lo + 40, 0:1],
                        Pcur[lo:lo + NB, TC:TC + 1],
                        cvec[lo:lo + NB, 0:1],
                        zb[lo:lo + NB, TC:TC + 1],
                        AOP.mult, AOP.add)
                # chunk-boundary values for the next columns' C reads, in
                # each destination chunk's scale frame (seed lanes stay 1.0)
                nc.vector.tensor_tensor(xs[:, 0:1], cvec[:], rb01[:],
                                        AOP.mult)
                # pass 2 fixup: x = z + carry * P'
                nc.vector.scalar_tensor_tensor(
                    xs[:, 1:TC + 1], Pcur[:, 1:TC + 1], cvec[:, 0:1],
                    zb[:, 1:TC + 1], AOP.mult, AOP.add)
                if s in (199, 200):
                    rcol = R199 if s == 199 else R200
                    nc.vector.scalar_tensor_tensor(
                        junk[:, 0:TC + 1], xs[:], 1.0,
                        aux[:, EM0:EM0 + TC + 1],
                        AOP.mult, AOP.mult, accum_out=sc[:, rcol:rcol + 1])
                if s == 193:
                    # lambda is final after the last epoch (col 191): move
                    # its readout matmul off the serial finalize tail
                    nc.vector.tensor_copy(lamF[:], sci[:, LAMI:LAMI + 1])
                    nc.vector.tensor_scalar_mul(lamF[:], lamF[:], LN2)
                    plam8 = psml.tile([NB, 512], F32, tag="prep")
                    nc.tensor.matmul(plam8[:, 0:1], aux[:, SEL0:SEL0 + NB],
                                     lamF[:], start=True, stop=True)
                    plam8_holder.append(plam8)
                if epoch:
                    nc.vector.tensor_reduce(
                        sc[:, MRE:MRE + 1], xs[:], mybir.AxisListType.X,
                        AOP.max, apply_absolute_value=True)
                    nc.vector.tensor_scalar_max(
                        sc[:, MRE:MRE + 1], sc[:, MRE:MRE + 1], 1.0)
                    # exponent-bit games: exact power-of-two rescale
                    nc.vector.tensor_scalar(
                        sci[:, RA:RA + 1], sc[:, MRE:MRE + 1].bitcast(INT32),
                        EXPMASK, None, AOP.bitwise_and)
                    nc.vector.tensor_scalar(
                        sci[:, RCPI:RCPI + 1], sci[:, RA:RA + 1],
                        -1, RCPBASE, AOP.mult, AOP.add)
                    rcpf = sci.bitcast(F32)[:, RCPI:RCPI + 1]
                    nc.vector.tensor_scalar_mul(xs[:], xs[:], rcpf)
                    nc.vector.tensor_scalar_mul(xm1[:], xm1[:], rcpf)
                    # restore the virtual-init seeds (DVE, not Pool memset,
                    # to avoid two cross-engine round trips mid-epoch)
                    nc.vector.tensor_scalar(xs[0:NB, 0:1], xs[0:NB, 0:1],
                                            0.0, 1.0, AOP.mult, AOP.add)
                    nc.vector.tensor_scalar(xm1[0:NB, 0:1], xm1[0:NB, 0:1],
                                            0.0, 1.0, AOP.mult, AOP.add)
                    nc.vector.tensor_scalar(
                        sci[:, EI:EI + 1], sci[:, RA:RA + 1],
                        23, None, AOP.logical_shift_right)
                    nc.vector.tensor_scalar(
                        sci[:, EI:EI + 1], sci[:, EI:EI + 1],
                        127, None, AOP.subtract)
                    nc.vector.tensor_tensor(sci[:, LAMI:LAMI + 1],
                                            sci[:, LAMI:LAMI + 1],
                                            sci[:, EI:EI + 1], AOP.add)
                    nc.vector.tensor_copy(sci[32:64, LSHI:LSHI + 1],
                                          sci[0:32, LAMI:LAMI + 1])
                    nc.vector.tensor_copy(sci[64:96, LSHI:LSHI + 1],
                                          sci[32:64, LAMI:LAMI + 1])
                    nc.vector.tensor_copy(sci[96:128, LSHI:LSHI + 1],
                                          sci[64:96, LAMI:LAMI + 1])
                    nc.vector.tensor_tensor(sci[:, DI:DI + 1],
                                            sci[:, LSHI:LSHI + 1],
                                            sci[:, LAMI:LAMI + 1],
                                            AOP.subtract)
                    nc.vector.tensor_scalar(sci[:, DI:DI + 1],
                                            sci[:, DI:DI + 1],
                                            126, -126, AOP.min, AOP.max)
                    nc.vector.tensor_scalar(sci[:, RB:RB + 1],
                                            sci[:, DI:DI + 1],
                                            127, None, AOP.add)
                    nc.vector.tensor_scalar(sci[:, RB:RB + 1],
                                            sci[:, RB:RB + 1],
                                            23, None, AOP.logical_shift_left)
                    # chunk-0 lanes must keep P' == 0 (no incoming carry)
                    nc.vector.tensor_scalar(sci[0:NB, RB:RB + 1],
                                            sci[0:NB, RB:RB + 1],
                                            0, None, AOP.mult)
                    # boundary-conversion vector: RB everywhere except the
                    # chunk-0 seed lanes, which stay exactly 1.0
                    nc.vector.tensor_copy(rb01[:], rbF[:, RB:RB + 1])
                    nc.vector.tensor_scalar(rb01[0:NB, 0:1], rb01[0:NB, 0:1],
                                            0.0, 1.0, AOP.mult, AOP.add)
                    emit_P(s + 1)       # uses the refreshed RB

        # ---- finalize ----
        nc.vector.tensor_tensor(sc[:, RTOT:RTOT + 1], sc[:, R199:R199 + 1],
                                sc[:, R200:R200 + 1], AOP.add)
        pr8 = psml.tile([NB, 512], F32, tag="psm")
        nc.tensor.matmul(pr8[:, 0:1], aux[:, SEL0:SEL0 + NB],
                         sc[:, RTOT:RTOT + 1], start=True, stop=True)
        plam8 = plam8_holder[0]
        nc.vector.tensor_tensor(v8[:, N8c:N8c + 1], n8parts[:, 0:1],
                                n8parts[:, 1:2], AOP.add)
        nc.vector.scalar_tensor_tensor(
            v8[:, N8c:N8c + 1], v8[:, N8c:N8c + 1], LN2,
            aux[0:NB, KB0:KB0 + 1], AOP.mult, AOP.add)
        # split r = m * 2^(E-127), m in [1,2): exact exponent, Ln on mantissa
        ri8 = pers.tile([NB, 2], INT32, tag="ri8")
        rf8 = pers.tile([NB, 2], F32, tag="rf8")
        nc.vector.tensor_scalar(ri8[:, 0:1], pr8[:, 0:1].bitcast(INT32),
                                23, None, AOP.logical_shift_right)
        nc.vector.tensor_copy(rf8[:, 0:1], ri8[:, 0:1])
        nc.vector.tensor_scalar(ri8[:, 1:2], pr8[:, 0:1].bitcast(INT32),
                                0x007FFFFF, 0x3F800000,
                                AOP.bitwise_and, AOP.bitwise_or)
        nc.scalar.activation(v8[:, LOGRc:LOGRc + 1],
                             ri8.bitcast(F32)[:, 1:2], AFT.Ln)
        nc.vector.tensor_scalar(rf8[:, 0:1], rf8[:, 0:1],
                                127.0, LN2, AOP.subtract, AOP.mult)
        nc.vector.tensor_tensor(v8[:, LOGRc:LOGRc + 1],
                                v8[:, LOGRc:LOGRc + 1],
                                rf8[:, 0:1], AOP.add)
        nc.vector.tensor_tensor(v8[:, T1c:T1c + 1], v8[:, LOGRc:LOGRc + 1],
                                v8[:, N8c:N8c + 1], AOP.add)
        nc.vector.scalar_tensor_tensor(
            v8[:, LOSSc:LOSSc + 1], v8[:, T1c:T1c + 1], -1.0, plam8[:, 0:1],
            AOP.mult, AOP.subtract)
        nc.sync.dma_start(loss_d[:], v8[:, LOSSc:LOSSc + 1])

    nc.finalize()
    return nc


def _host_prep(y_true, y_pred, input_lengths, label_lengths):
    bf16 = mybir.dt.np(BF16)
    in_maps = []
    for core in range(NCORE):
        bsl = slice(core * NB, (core + 1) * NB)
        yt = y_true[bsl]
        ilen = input_lengths[bsl].astype(np.int64)
        llen = label_lengths[bsl].astype(np.int64)

        big = np.zeros((C, BIGN), np.float32)
        big[:, YP0:YP0 + NB * T] = (
            y_pred[bsl].transpose(2, 0, 1).reshape(C, NB * T))
        aux = np.zeros((128, AUXN), np.float32)

        for b in range(NB):
            l = int(llen[b]); o = 200 - 2 * l
            ext = np.full(S, -1, np.int32)
            for k in range(2 * l + 1):
                ext[o + k] = C - 1 if k % 2 == 0 else yt[b, (k - 1) // 2]
            gb = np.zeros((C, S), np.float32)
            for s in range(S):
                if ext[s] >= 0:
                    gb[ext[s], s] = 1.0
                k = s - o
                if k >= 2 and k % 2 == 1 and ext[s] != ext[s - 2]:
                    for c in range(NCH):
                        aux[32 * c + b, SKP0 + s] = 1.0
            big[:, G0 + b * S:G0 + (b + 1) * S] = gb
            wvec = gb.sum(axis=1) * np.float32(KAPPA * np.sqrt(2.0) / (2 * l + 1))
            big[:, W0 + b] = wvec
            big[b, SELB0 + b * 128:SELB0 + (b + 1) * 128] = 1.0
            big[:, WB0 + b * NB + b] = wvec
            for sig in range(2):
                s0, ps = (0, 128) if sig == 0 else (128, S - 128)
                for sp in range(ps):
                    if ext[s0 + sp] >= 0:
                        aux[sp, VE0 + NB * sig + b] = 1e-7
                for tgt in (o, o + 1):
                    if s0 <= tgt < s0 + ps:
                        aux[tgt - s0, I00 + NB * sig + b] = 1.0
            tstar = int(ilen[b]) - 1
            cstar = tstar // TC
            jstar = tstar - cstar * TC + 1
            aux[32 * cstar + b, EM0 + jstar] = 1.0
            aux[32 * cstar + b, SEL0 + b] = 1.0
            aux[b, KB0] = -np.log(2.0) * 127.0 * (tstar + 1)
            aux[b, TM0:TM0 + tstar + 1] = 1.0

        in_maps.append({"big": big.astype(bf16), "aux": aux})
    return in_maps


def kernel(y_true, y_pred, input_lengths, label_lengths):
    y_true = np.asarray(y_true)
    y_pred = np.asarray(y_pred, dtype=np.float32)
    input_lengths = np.asarray(input_lengths)
    label_lengths = np.asarray(label_lengths)

    if "nc" not in _cached:
        _cached["nc"] = _build_program()
    nc = _cached["nc"]

    in_maps = _host_prep(y_true, y_pred, input_lengths, label_lengths)
    res = run_bass_kernel_spmd(nc, in_maps, core_ids=list(range(NCORE)))
    out = np.concatenate([res.results[i]["loss"] for i in range(NCORE)], axis=0)
    return out.astype(np.float32)
